# revision 34
# baseline (speedup 1.0000x reference)
"""Trainium2 Bass kernel for nn_CGCoupler (segment_reduce).

The CG coupling tables have a fixed block structure: every index triple
(repids_in1, repids_in2, repids_out) decomposes into 147 block-ops

    out[:, bo*64:(bo+1)*64] += c * x1[:, b1*64:(b1+1)*64] * x2[:, b2*64:(b2+1)*64]

with 64-aligned contiguous blocks and a scalar coefficient c per op (the
cg_tilde rows are constant within each 64-block). Engine split:

  - Act:  fp32 -> fp16 casts of the x tiles + PSUM -> SBUF output copy.
  - DVE:  the 70 distinct (b1,b2) pair products, batched into ~10
          affine-line tensor_mul instructions (fp16 2x mode).
  - PE:   scale-by-c + segment-reduce as matmuls with c*I128 stationary
          weights (host-built, DMA'd) accumulating into PSUM; ops are
          batched into runs with constant (slot, bo) strides so one matmul
          covers up to 8 ops; runs covering every bo exactly once run
          first with start=True, the rest accumulate.

Data-parallel over rows: 4096 rows -> 8 cores x 512 rows.
"""
import numpy as np

N_CORES = 8
ROWS_PER_CORE = 512
D = 1024
NB = 16       # 64-wide blocks per tensor
MAX_RUN = 8   # 8 ops * 64 lanes = 512 moving columns (PE limit)

_CACHE = {}


def _dedup_ldweights(m):
    """Drop InstLdweights that reload the exact weights already resident in
    the PE array (same memref/offset/ap as the previous load in the block,
    no sync attached). Matmuls between loads don't disturb the array, so
    consecutive same-weight matmuls only need the first load."""
    removed = 0
    for f in m.functions:
        for bb in f.blocks:
            insts = list(bb.instructions)
            out = []
            prev_sig = None
            for ins in insts:
                if type(ins).__name__ == "InstLdweights":
                    w = ins.ins[0]
                    sig = (getattr(w, "memref", None), getattr(w, "offset", 0),
                           str(getattr(w, "ap", None)))
                    si = ins.sync_info
                    clean = si is None or (not si.on_wait and not si.on_update)
                    if sig == prev_sig and clean:
                        removed += 1
                        continue
                    prev_sig = sig
                out.append(ins)
            bb.instructions = out
    return removed


def _coalesce_matmul_sem_updates(m):
    """Matmuls each carry a sem-inc that forces a pipeline drain before the
    update publishes. All waits on those sems sit at row-tile boundaries
    (cumulative counts), so move the increments onto the matmuls that hit a
    waited threshold; intermediate matmuls then chain back-to-back."""
    import concourse.mybir as mybir
    from collections import defaultdict
    for f in m.functions:
        all_insts = [ins for bb in f.blocks for ins in bb.instructions]
        thresholds = defaultdict(set)
        regwaited = set()
        mmsems, other_upd = set(), set()
        for ins in all_insts:
            si = ins.sync_info
            if not si:
                continue
            for w in si.on_wait:
                if w.wait_mode == "sem-ge-imm" and w.wait_value is not None:
                    thresholds[w.id].add(w.wait_value)
                else:
                    regwaited.add(w.id)
            for u in si.on_update:
                if (type(ins).__name__ == "InstMatmult"
                        and u.update_mode == "sem-inc"):
                    mmsems.add(u.id)
                else:
                    other_upd.add(u.id)
        mmsems -= other_upd
        mmsems -= regwaited
        # dry-run: every waited threshold must land exactly on a cumulative
        # matmul count, else leave that sem alone
        cum = defaultdict(int)
        hit = defaultdict(set)
        last_mm = {}
        for i, ins in enumerate(all_insts):
            si = ins.sync_info
            if not si or type(ins).__name__ != "InstMatmult":
                continue
            for u in si.on_update:
                if u.id in mmsems and u.update_mode == "sem-inc":
                    cum[u.id] += u.update_value
                    hit[u.id].add(cum[u.id])
                    last_mm[u.id] = i
        ok = {s for s in mmsems if thresholds[s] <= hit[s]}
        # publish points: the matmuls whose cumulative count hits a waited
        # threshold (or the final matmul). Updates stay value-1 (walrus
        # requires it); waiters are renumbered to the publish-point rank.
        publish_at = defaultdict(set)   # sem -> set of old cumulative counts
        for s in ok:
            publish_at[s] = {t for t in hit[s]
                             if t in thresholds[s] or t == cum[s]}
        count = defaultdict(int)
        for i, ins in enumerate(all_insts):
            si = ins.sync_info
            if not si or type(ins).__name__ != "InstMatmult":
                continue
            new_upd, changed = [], False
            for u in si.on_update:
                if u.id in ok and u.update_mode == "sem-inc":
                    count[u.id] += u.update_value
                    if count[u.id] in publish_at[u.id]:
                        new_upd.append(u)
                    else:
                        changed = True
                else:
                    new_upd.append(u)
            if changed:
                ins.sync_info = mybir.SyncInfo(
                    on_wait=list(si.on_wait), on_update=new_upd)
        # renumber every wait on the coalesced sems
        for ins in all_insts:
            si = ins.sync_info
            if not si:
                continue
            if not any(w.id in ok and w.wait_mode == "sem-ge-imm"
                       for w in si.on_wait):
                continue
            new_waits = []
            for w in si.on_wait:
                if w.id in ok and w.wait_mode == "sem-ge-imm":
                    rank = sum(1 for t in publish_at[w.id]
                               if t <= w.wait_value)
                    new_waits.append(mybir.SyncWait(
                        sync_type=w.sync_type, id=w.id, ant_name=w.ant_name,
                        wait_mode="sem-ge-imm", wait_value=rank,
                        wait_reg=None))
                else:
                    new_waits.append(w)
            ins.sync_info = mybir.SyncInfo(
                on_wait=new_waits, on_update=list(si.on_update))


def _extract_ops(cg, r1, r2, ro):
    """Recover (b1, b2, bo, c) per 64-wide op block; assert the structure."""
    k = cg.shape[0]
    assert k % 64 == 0
    n_ops = k // 64
    r1 = r1.reshape(n_ops, 64)
    r2 = r2.reshape(n_ops, 64)
    ro = ro.reshape(n_ops, 64)
    cg = cg.reshape(n_ops, 64)
    lane = np.arange(64)
    assert np.all(r1 == r1[:, :1] + lane) and np.all(r2 == r2[:, :1] + lane)
    assert np.all(ro == ro[:, :1] + lane)
    assert np.all(r1[:, 0] % 64 == 0) and np.all(r2[:, 0] % 64 == 0)
    assert np.all(ro[:, 0] % 64 == 0)
    assert np.all(cg == cg[:, :1])
    b1 = (r1[:, 0] // 64).astype(int)
    b2 = (r2[:, 0] // 64).astype(int)
    bo = (ro[:, 0] // 64).astype(int)
    c = cg[:, 0].astype(np.float64)
    return list(zip(b1.tolist(), b2.tolist(), bo.tolist(), c.tolist()))


def _cover_pairs(pairs):
    """Cover the (b1,b2) pair set with affine lines, preferring axis-aligned
    directions (they merge into long contiguous runs for this family of
    tables). Returns [(pair_list, d1, d2)]."""
    left = set(pairs)
    lines = []
    for d in [(0, 1), (1, 0), (1, -1), (1, 1)]:
        while True:
            best = None
            for p in sorted(left):
                if (p[0] - d[0], p[1] - d[1]) in left:
                    continue  # not a line start
                run = [p]
                q = (p[0] + d[0], p[1] + d[1])
                while q in left:
                    run.append(q)
                    q = (q[0] + d[0], q[1] + d[1])
                if best is None or len(run) > len(best):
                    best = run
            if best is None or len(best) < 2:
                break
            lines.append((best, d[0], d[1]))
            left -= set(best)
    while left:  # leftovers: greedy longest any-direction
        best = None
        for p in sorted(left):
            for d1 in range(-15, 16):
                for d2 in range(-15, 16):
                    if d1 == 0 and d2 == 0:
                        continue
                    run = [p]
                    q = (p[0] + d1, p[1] + d2)
                    while q in left:
                        run.append(q)
                        q = (q[0] + d1, q[1] + d2)
                    if best is None or len(run) > len(best[0]):
                        best = (run, d1, d2)
        run, d1, d2 = best
        if len(run) == 1:
            d1 = d2 = 0
        lines.append((run, d1, d2))
        left -= set(run)
    return lines


def _find_runs(group, allow_dbo0):
    """Cover a c-group's ops [(slot, bo)] with maximal double-arithmetic
    chains (slot and bo strides constant, dbo >= 0, length <= MAX_RUN).
    Returns [(s0, ds, bo0, dbo, L)]."""
    left = set(range(len(group)))
    idx = {v: k for k, v in enumerate(group)}
    runs = []
    while left:
        best = None
        for i in sorted(left):
            s0, o0 = group[i]
            for j in sorted(left):
                if j == i:
                    continue
                ds = group[j][0] - s0
                do = group[j][1] - o0
                if do < 0 or (do == 0 and (not allow_dbo0 or ds <= 0)):
                    continue
                chain = [i, j]
                ns, no = group[j][0] + ds, group[j][1] + do
                while (len(chain) < MAX_RUN and (ns, no) in idx
                       and idx[(ns, no)] in left and idx[(ns, no)] not in chain):
                    chain.append(idx[(ns, no)])
                    ns += ds
                    no += do
                if best is None or len(chain) > len(best[0]):
                    best = (chain, ds, do)
        if best is None or len(best[0]) < 2:
            for i in sorted(left):
                runs.append((group[i][0], 0, group[i][1], 0, 1))
            break
        chain, ds, do = best
        s0, o0 = group[chain[0]]
        runs.append((s0, ds, o0, do, len(chain)))
        left -= set(chain)
    return runs


def _analyze(cg, r1, r2, ro, allow_dbo0=True):
    """Derive the full kernel structure from the runtime tables."""
    ops = _extract_ops(cg, r1, r2, ro)
    pairs = sorted(set((b1, b2) for b1, b2, _, _ in ops))
    lines = _cover_pairs(pairs)
    slot = {}
    a_lines = []
    for run, d1, d2 in lines:
        a_lines.append((len(slot), len(run), run[0][0], run[0][1], d1, d2))
        for p in run:
            slot[p] = len(slot)
    # merge consecutive b2-rows (d=(0,1), same b2 span, b1 step 1, contiguous
    # slots) into one 2D-family instruction with a broadcast AP
    a_instrs = []
    i = 0
    while i < len(a_lines):
        s0, L, a1, a2, d1, d2 = a_lines[i]
        j = i + 1
        if (d1, d2) == (0, 1):
            while j < len(a_lines):
                t0, tL, t1, t2, td1, td2 = a_lines[j]
                if ((td1, td2) == (0, 1) and tL == L and t2 == a2
                        and t1 == a1 + (j - i) and t0 == s0 + (j - i) * L):
                    j += 1
                else:
                    break
        if j - i > 1:
            a_instrs.append(("fam", s0, j - i, L, a1, a2))
        else:
            a_instrs.append(("line", s0, L, a1, a2, d1, d2))
        i = j

    cvals = sorted(set(c for _, _, _, c in ops))
    gidx = {c: i for i, c in enumerate(cvals)}
    bygroup = {}
    for b1, b2, bo, c in ops:
        bygroup.setdefault(gidx[c], []).append((slot[(b1, b2)], bo))

    # psum is zeroed by two full-bank start=True matmuls against zero
    # weights (a start=True matmul resets accumulation state at bank
    # granularity, so per-region "opener" flags are not safe); every real
    # run pure-accumulates, ordered by weight group to batch ldweights
    mm = []  # (g, s0, ds, bo0, dbo, L, start=False)
    for g in sorted(bygroup):
        for r in _find_runs(sorted(bygroup[g]), allow_dbo0):
            mm.append((g, *r, False))
    return {
        "a_instrs": a_instrs,
        "n_pairs": len(pairs),
        "cvals": np.array(cvals, dtype=np.float64),
        "mm": mm,
    }


def _build(struct):
    from concourse import bacc, mybir
    import concourse.tile as tile
    from concourse.masks import make_identity

    f32 = mybir.dt.float32
    f16 = mybir.dt.float16
    P = struct["n_pairs"]
    NG = len(struct["cvals"])
    nc = bacc.Bacc("TRN2", target_bir_lowering=False)
    x1_d = nc.dram_tensor("x1", [ROWS_PER_CORE, D], f32, kind="ExternalInput")
    x2_d = nc.dram_tensor("x2", [ROWS_PER_CORE, D], f32, kind="ExternalInput")
    cv_d = nc.dram_tensor("cvals16", [1, NG + 1], f16, kind="ExternalInput")
    out_d = nc.dram_tensor("out", [ROWS_PER_CORE, D], f32, kind="ExternalOutput")

    with tile.TileContext(nc) as tc:
        with (
            tc.tile_pool(name="const", bufs=1) as constp,
            tc.tile_pool(name="io", bufs=2) as iop,
            tc.tile_pool(name="spp", bufs=2) as spp,
            tc.psum_pool(name="psp", bufs=2) as psp,
        ):
            # build the NG scaled identities + one zero block on-device:
            # W[p, g*128 + m] = (p == m) ? c_g : 0
            cv = constp.tile([1, NG + 1], f16)
            nc.sync.dma_start(cv[:], cv_d[:])
            cvrep = constp.tile([128, NG + 1], f16)
            nc.gpsimd.partition_broadcast(cvrep[:], cv[:])
            ident = constp.tile([128, 128], f16)
            make_identity(nc, ident[:])
            W = constp.tile([128, (NG + 1) * 128], f16)
            W3 = W[:].rearrange("p (g m) -> p g m", g=NG + 1)
            cvb = cvrep[:].rearrange("p (g one) -> p g one", one=1)
            idb = ident[:].rearrange("p (one m) -> p one m", one=1)
            nc.vector.tensor_mul(W3, idb.to_broadcast([128, NG + 1, 128]),
                                 cvb.to_broadcast([128, NG + 1, 128]))

            def bsl(ap3, b0, d, k):
                if k == 1:
                    return ap3[:, b0:b0 + 1, :]
                if d == 0:
                    return ap3[:, b0:b0 + 1, :].to_broadcast([128, k, 64])
                if d > 0:
                    return ap3[:, b0:b0 + (k - 1) * d + 1:d, :]
                stop = b0 + (k - 1) * d - 1
                return ap3[:, b0:(stop if stop >= 0 else None):d, :]

            pending = None  # (psum tile, row offset) awaiting copy-out
            for rt in range(ROWS_PER_CORE // 128):
                r0 = rt * 128
                x1t = iop.tile([128, D], f32, tag="x1t")
                x2t = iop.tile([128, D], f32, tag="x2t")
                nc.sync.dma_start(x1t[:], x1_d[r0:r0 + 128])
                nc.sync.dma_start(x2t[:], x2_d[r0:r0 + 128])

                # fp32 -> fp16 casts on the otherwise-idle Act engine
                x1h = iop.tile([128, D], f16, tag="x1h")
                x2h = iop.tile([128, D], f16, tag="x2h")
                nc.scalar.copy(x1h[:], x1t[:])
                nc.vector.tensor_copy(x2h[:], x2t[:])

                x13 = x1h[:].rearrange("p (b n) -> p b n", b=NB)
                x23 = x2h[:].rearrange("p (b n) -> p b n", b=NB)
                x14 = x1h[:].rearrange("p (b one n) -> p b one n", b=NB, one=1)
                x24 = x2h[:].rearrange("p (one b n) -> p one b n", one=1, b=NB)

                # pass A: distinct pair products; affine lines plus merged
                # 2D families (b1-range x b2-range) via broadcast APs
                sp = spp.tile([128, P * 64], f16, tag="sp")
                sp3 = sp[:].rearrange("p (q n) -> p q n", q=P)
                for ai in struct["a_instrs"]:
                    if ai[0] == "fam":
                        _, s0, n1, n2, a1, a2 = ai
                        out4 = sp3[:, s0:s0 + n1 * n2, :].rearrange(
                            "p (a b) n -> p a b n", a=n1)
                        nc.vector.tensor_mul(
                            out4,
                            x14[:, a1:a1 + n1, :, :].to_broadcast([128, n1, n2, 64]),
                            x24[:, :, a2:a2 + n2, :].to_broadcast([128, n1, n2, 64]))
                    else:
                        _, s0, L, a1, a2, d1, d2 = ai
                        nc.vector.tensor_mul(sp3[:, s0:s0 + L, :],
                                             bsl(x13, a1, d1, L),
                                             bsl(x23, a2, d2, L))

                # scale + segment-reduce on PE, batched into strided runs.
                # two start=True matmuls against the zero weight block reset
                # each psum bank, then every run accumulates
                ps = psp.tile([128, D], f32, tag="ps")
                ps3 = ps[:].rearrange("p (o n) -> p o n", o=NB)
                zg = NG  # zero weight block appended after the NG real ones
                for half in range(2):
                    nc.tensor.matmul(ps[:, half * 512:(half + 1) * 512],
                                     W[:, zg * 128:(zg + 1) * 128],
                                     W[:, 0:512],
                                     start=True, stop=True,
                                     skip_group_check=True)
                for (g, s0, ds, bo0, dbo, L, start) in struct["mm"]:
                    rhs = bsl(sp3, s0, ds, L)
                    out = bsl(ps3, bo0, dbo, L)
                    nc.tensor.matmul(out, W[:, g * 128:(g + 1) * 128], rhs,
                                     start=False, stop=True,
                                     skip_group_check=True)

                # software pipelining: copy the *previous* tile's psum out
                # here so this tile's casts/A aren't queued behind a copy
                # that waits on a full matmul phase (engine streams execute
                # in emission order)
                if pending is not None:
                    pps, pr0 = pending
                    outt = iop.tile([128, D], f32, tag="outt")
                    nc.scalar.copy(outt[:], pps[:])
                    nc.sync.dma_start(out_d[pr0:pr0 + 128], outt[:])
                pending = (ps, r0)
            pps, pr0 = pending
            outt = iop.tile([128, D], f32, tag="outt")
            nc.scalar.copy(outt[:], pps[:])
            nc.sync.dma_start(out_d[pr0:pr0 + 128], outt[:])

    nc.compile()
    _dedup_ldweights(nc.m)
    _coalesce_matmul_sem_updates(nc.m)
    return nc


def _get_nc(struct=None, key=None):
    if "nc" not in _CACHE:
        _CACHE["nc"] = _build(struct)
    return _CACHE["nc"]


def _make_in_maps(np_inputs):
    x1 = np.ascontiguousarray(np.asarray(np_inputs["x1"], dtype=np.float32))
    x2 = np.ascontiguousarray(np.asarray(np_inputs["x2"], dtype=np.float32))
    struct = _analyze(np.asarray(np_inputs["cg_tilde"], dtype=np.float32),
                      np.asarray(np_inputs["repids_in1"]),
                      np.asarray(np_inputs["repids_in2"]),
                      np.asarray(np_inputs["repids_out"]))
    NG = len(struct["cvals"])
    cv16 = np.zeros((1, NG + 1), dtype=np.float16)  # last entry: zero block
    cv16[0, :NG] = struct["cvals"].astype(np.float16)
    n = x1.shape[0]
    rows = n // N_CORES
    in_maps = []
    for k in range(N_CORES):
        sl = slice(k * rows, (k + 1) * rows)
        in_maps.append({
            "x1": np.ascontiguousarray(x1[sl]),
            "x2": np.ascontiguousarray(x2[sl]),
            "cvals16": cv16,
        })
    return struct, in_maps


def kernel(x1, x2, cg_tilde, repids_in1, repids_in2, repids_out, out_dim):
    from concourse.bass_utils import run_bass_kernel_spmd

    np_inputs = {"x1": x1, "x2": x2, "cg_tilde": cg_tilde,
                 "repids_in1": repids_in1, "repids_in2": repids_in2,
                 "repids_out": repids_out}
    struct, in_maps = _make_in_maps(np_inputs)
    nc = _get_nc(struct)
    res = run_bass_kernel_spmd(nc, in_maps, core_ids=list(range(N_CORES)))
    out = np.concatenate([res.results[k]["out"] for k in range(N_CORES)], axis=0)
    return out
